# revision 5
# baseline (speedup 1.0000x reference)
"""Masked batched dot-product attention on 8 Trainium2 NeuronCores (Bass/Tile).

Reference computation (per batch b):
    scores = Q @ K^T / sqrt(D)                  [Q, K]
    scores[:, k >= valid_len[b]] = -1e6
    attn   = softmax(scores, axis=-1)
    out    = attn @ V                           [Q, V]

Strategy:
  - Data-parallel over the batch dim: 32 batches -> 8 cores x 4 slots.
    Batches are assigned to (slot, core) sorted by valid_len so that all
    cores run the same (SPMD) trace while each slot's K-extent is trimmed
    to the slot-wise max number of 128-wide K chunks.
  - Per (slot, chunk): scoresT[k, q] = KT_chunk.T @ QT on PE (float32r),
    exp via ScalarE with a per-partition additive mask bias (-1e9 on
    k >= valid_len -> exp == 0), then two accumulating PE matmuls over
    chunks: O^T[v, q] += V_chunk.T-contraction and sums[1, q] += ones
    contraction (softmax denominators).
  - exp is numerically safe without max-subtraction: scores/sqrt(D) is
    ~N(0,1) here, and the reference's -1e6 mask also underflows to exactly
    0 after exp in f32.
  - O^T and sums are written back; the final transpose to [q, v] and the
    divide by sums happen on the host during unsharding.
"""

import math

import numpy as np

import concourse.tile as tile
import concourse.mybir as mybir
from concourse import bacc
from concourse.bass_utils import run_bass_kernel_spmd

F32 = mybir.dt.float32
F32R = mybir.dt.float32r

B, Q, K, D, V = 32, 1024, 1024, 128, 128
N_CORES = 8
S = B // N_CORES          # batch slots per core
CH = 128                  # K-chunk size (PE contraction width)
NCH = K // CH             # max chunks
HALF = 512                # fp32 moving-operand limit per matmul
SCALE = 1.0 / math.sqrt(D)
NEG_BIAS = -1.0e9


def _build(n_chunks):
    """Build + compile the SPMD bass module for per-slot chunk counts."""
    nc = bacc.Bacc("TRN2", target_bir_lowering=False, debug=False,
                   num_devices=N_CORES)
    qt = nc.dram_tensor("qt", [S, D, Q], F32R, kind="ExternalInput")
    kt = nc.dram_tensor("kt", [S, D, K], F32R, kind="ExternalInput")
    vv = nc.dram_tensor("v", [S, K, V], F32R, kind="ExternalInput")
    mb = nc.dram_tensor("mbias", [S, CH, NCH], F32, kind="ExternalInput")
    on = nc.dram_tensor("ones", [CH, 1], F32R, kind="ExternalInput")
    ot = nc.dram_tensor("ot", [S, V, Q], F32, kind="ExternalOutput")
    sm = nc.dram_tensor("sums", [S, Q], F32, kind="ExternalOutput")

    with tile.TileContext(nc) as tc:
        with (
            tc.tile_pool(name="io", bufs=2) as io,
            tc.tile_pool(name="consts", bufs=1) as consts,
            tc.tile_pool(name="expp", bufs=3) as expp,
            tc.tile_pool(name="outp", bufs=2) as outp,
            tc.tile_pool(name="ps_sc", bufs=2, space="PSUM") as ps_sc_pool,
            tc.tile_pool(name="ps_ot", bufs=1, space="PSUM") as ps_ot_pool,
            tc.tile_pool(name="ps_sm", bufs=1, space="PSUM") as ps_sm_pool,
        ):
            ones_t = consts.tile([CH, 1], F32R)
            nc.sync.dma_start(out=ones_t, in_=on.ap())
            bias_t = consts.tile([CH, S, NCH], F32)
            nc.sync.dma_start(out=bias_t, in_=mb.ap().rearrange("s p j -> p s j"))

            for s in range(S):
                n_c = n_chunks[s]
                sb_qt = io.tile([D, Q], F32R, tag="qt")
                nc.sync.dma_start(out=sb_qt, in_=qt.ap()[s])
                sb_kt = io.tile([D, n_c * CH], F32R, tag="kt")
                nc.sync.dma_start(out=sb_kt, in_=kt.ap()[s, :, 0:n_c * CH])
                sb_v = io.tile([CH, n_c, V], F32R, tag="v")
                nc.sync.dma_start(
                    out=sb_v,
                    in_=vv.ap()[s, 0:n_c * CH, :].rearrange("(c p) v -> p c v", p=CH),
                )

                ps_ot = ps_ot_pool.tile([V, Q], F32, tag="ot")
                ps_sums = ps_sm_pool.tile([1, 2, HALF], F32, tag="sums")
                for j in range(n_c):
                    ps_scores = ps_sc_pool.tile([CH, Q], F32, tag="sc")
                    ktj = sb_kt[:, j * CH:(j + 1) * CH]
                    for h in range(2):
                        nc.tensor.matmul(
                            ps_scores[:, h * HALF:(h + 1) * HALF],
                            lhsT=ktj,
                            rhs=sb_qt[:, h * HALF:(h + 1) * HALF],
                            start=True, stop=True,
                        )
                    sb_exp = expp.tile([CH, Q], F32R, tag="exp")
                    nc.scalar.activation(
                        sb_exp, ps_scores,
                        func=mybir.ActivationFunctionType.Exp,
                        bias=bias_t[:, s, j:j + 1],
                        scale=SCALE,
                    )
                    e_r = sb_exp
                    vj = sb_v[:, j, :]
                    first, last = (j == 0), (j == n_c - 1)
                    for h in range(2):
                        nc.tensor.matmul(
                            ps_ot[:, h * HALF:(h + 1) * HALF],
                            lhsT=vj,
                            rhs=e_r[:, h * HALF:(h + 1) * HALF],
                            start=first, stop=last,
                        )
                    for h in range(2):
                        nc.tensor.matmul(
                            ps_sums[:, h, :],
                            lhsT=ones_t,
                            rhs=e_r[:, h * HALF:(h + 1) * HALF],
                            start=first, stop=last,
                        )
                sb_ot = outp.tile([V, Q], F32, tag="ot")
                nc.vector.tensor_copy(sb_ot, ps_ot)
                nc.sync.dma_start(out=ot.ap()[s], in_=sb_ot)
                sb_sums = outp.tile([1, Q], F32, tag="sums")
                nc.vector.tensor_copy(sb_sums, ps_sums.rearrange("p a b -> p (a b)"))
                nc.sync.dma_start(out=sm.ap()[s:s + 1, :], in_=sb_sums)
    nc.compile()
    return nc


_MODULE_CACHE = {}


def _get_module(n_chunks):
    key = tuple(n_chunks)
    if key not in _MODULE_CACHE:
        _MODULE_CACHE[key] = _build(key)
    return _MODULE_CACHE[key]


def _plan(L):
    """Assign batches to (slot, core) sorted by valid_len; per-slot chunk count."""
    order = np.argsort(L, kind="stable")
    grid = order.reshape(S, N_CORES)       # grid[s, c] = batch index
    n_chunks = tuple(
        max(1, int(math.ceil(int(L[grid[s, -1]]) / CH))) for s in range(S)
    )
    return grid, n_chunks


def _prepare_inputs(q, k, v, L, grid):
    kidx = np.arange(K).reshape(NCH, CH).T      # [CH, NCH] k index per (p, chunk)
    in_maps = []
    for c in range(N_CORES):
        bs = grid[:, c]
        qt_c = np.ascontiguousarray(q[bs].transpose(0, 2, 1))
        kt_c = np.ascontiguousarray(k[bs].transpose(0, 2, 1))
        v_c = np.ascontiguousarray(v[bs])
        mb_c = np.empty((S, CH, NCH), np.float32)
        for s in range(S):
            mb_c[s] = np.where(kidx < int(L[grid[s, c]]), 0.0, NEG_BIAS)
        in_maps.append({
            "qt": qt_c, "kt": kt_c, "v": v_c, "mbias": mb_c,
            "ones": np.ones((CH, 1), np.float32),
        })
    return in_maps


def _postprocess(results, grid):
    out = np.empty((B, Q, V), np.float32)
    for c in range(N_CORES):
        otc = results[c]["ot"]
        smc = results[c]["sums"]
        for s in range(S):
            b = grid[s, c]
            out[b] = otc[s].T / smc[s][:, None]
    return out


def kernel(**inputs):
    q = np.ascontiguousarray(np.asarray(inputs["queries"], dtype=np.float32))
    k = np.ascontiguousarray(np.asarray(inputs["keys"], dtype=np.float32))
    v = np.ascontiguousarray(np.asarray(inputs["values"], dtype=np.float32))
    L = np.clip(np.asarray(inputs["valid_lens"]).astype(np.int64).reshape(-1), 1, K)
    grid, n_chunks = _plan(L)
    nc = _get_module(n_chunks)
    in_maps = _prepare_inputs(q, k, v, L, grid)
    res = run_bass_kernel_spmd(nc, in_maps, core_ids=list(range(N_CORES)))
    return _postprocess(res.results, grid)


# revision 30
# speedup vs baseline: 419.2360x; 419.2360x over previous
"""Masked batched dot-product attention on 8 Trainium2 NeuronCores (Bass/Tile).

Reference computation (per batch b):
    scores = Q @ K^T / sqrt(D)                  [Q, K]
    scores[:, k >= valid_len[b]] = -1e6
    attn   = softmax(scores, axis=-1)
    out    = attn @ V                           [Q, V]

Strategy:
  - Data-parallel over the batch dim: 32 batches -> 8 cores x 4 slots.
    Batches are assigned to (slot, core) sorted by valid_len so that all
    cores run the same (SPMD) trace while each slot's K-extent is trimmed
    to the slot-wise max number of 128-wide K chunks.
  - Per (slot, chunk): scoresT[k, q] = KT_chunk.T @ QT on PE (float32r),
    exp via ScalarE with a per-partition additive mask bias (-1e9 on
    k >= valid_len -> exp == 0), then two accumulating PE matmuls over
    chunks: O^T[v, q] += V_chunk.T-contraction and sums[1, q] += ones
    contraction (softmax denominators).
  - exp is numerically safe without max-subtraction: scores/sqrt(D) is
    ~N(0,1) here, and the reference's -1e6 mask also underflows to exactly
    0 after exp in f32.
  - O^T and sums are written back; the final transpose to [q, v] and the
    divide by sums happen on the host during unsharding.
"""

import math

import ml_dtypes
import numpy as np

import concourse.tile as tile
import concourse.mybir as mybir
from concourse import bacc
from concourse.bass_utils import run_bass_kernel_spmd

F32 = mybir.dt.float32
F32R = mybir.dt.float32r
BF16 = mybir.dt.bfloat16

B, Q, K, D, V = 32, 1024, 1024, 128, 128
N_CORES = 8
S = B // N_CORES          # batch slots per core
CH = 128                  # K-chunk size (PE contraction width)
NCH = K // CH             # max chunks
HALF = 512                # fp32 moving-operand limit per matmul
SCALE = 1.0 / math.sqrt(D)
NEG_BIAS = -1.0e9


def _build(n_chunks):
    """Build + compile the SPMD bass module for per-slot chunk counts."""
    nc = bacc.Bacc("TRN2", target_bir_lowering=False, debug=False,
                   num_devices=N_CORES)
    qt = nc.dram_tensor("qt", [S, D, Q], BF16, kind="ExternalInput")
    kt = nc.dram_tensor("kt", [S, D, K], BF16, kind="ExternalInput")
    vv = nc.dram_tensor("v", [S, K, V], BF16, kind="ExternalInput")
    mb = nc.dram_tensor("mbias", [S, CH, NCH + 1], F32, kind="ExternalInput")
    ot = nc.dram_tensor("ot", [S, V, Q], BF16, kind="ExternalOutput")
    am = nc.dram_tensor("acc", [S, CH, Q], BF16, kind="ExternalOutput")

    with tile.TileContext(nc) as tc:
        with (
            tc.tile_pool(name="io", bufs=3) as io,
            tc.tile_pool(name="iov", bufs=4) as iov,
            tc.tile_pool(name="consts", bufs=1) as consts,
            tc.tile_pool(name="expp", bufs=6) as expp,
            tc.tile_pool(name="accp", bufs=2) as accp,
            tc.tile_pool(name="outp", bufs=2) as outp,
            tc.tile_pool(name="ps_sc", bufs=3, space="PSUM") as ps_sc_pool,
            tc.tile_pool(name="ps_ot", bufs=1, space="PSUM") as ps_ot_pool,
        ):
            bias_t = consts.tile([CH, S, NCH + 1], F32)
            nc.gpsimd.dma_start(out=bias_t, in_=mb.ap().rearrange("s p j -> p s j"))
            # Warm the PE clock (HAM) with dependency-free dummy matmuls so the
            # first real matmuls run at full rate once input DMAs land.
            warm_w = consts.tile([CH, 1], BF16)
            nc.vector.memset(warm_w, 0.0)
            warm_x = consts.tile([CH, 256], BF16)
            nc.vector.memset(warm_x, 0.0)
            ps_warm = ps_ot_pool.tile([1, 256], F32, tag="ot", name="ps_warm")
            for _ in range(9):
                nc.tensor.matmul(ps_warm, lhsT=warm_w, rhs=warm_x,
                                 start=True, stop=True)
            # Pre-load the Exp LUT table set so the first real exp skips it.
            warm_e = consts.tile([CH, 1], BF16)
            nc.scalar.activation(warm_e, warm_x[:, 0:1],
                                 func=mybir.ActivationFunctionType.Exp)

            _a = sorted(range(S), key=lambda i: n_chunks[i])
            slot_order = _a[1:] + _a[:1]
            # All V loads up-front on the Pool/SWDGE ring so later (possibly
            # blocked) output DMAs on that ring never delay them.
            sb_vs = {}
            for s in slot_order:
                n_c = n_chunks[s]
                sb_v = iov.tile([CH, n_c, V], BF16, tag=f"v{s}", name=f"v{s}")
                nc.gpsimd.dma_start(
                    out=sb_v,
                    in_=vv.ap()[s, 0:n_c * CH, :].rearrange("(c p) v -> p c v", p=CH),
                )
                sb_vs[s] = sb_v
            for s in slot_order:
                n_c = n_chunks[s]
                sb_v = sb_vs[s]
                # Q half 0 first (first matmul operand), then KT, then Q half 1.
                sb_qt = []
                for h in range(2):
                    qt_h = io.tile([D, HALF], BF16, tag=f"qt{h}", name=f"qt{h}")
                    nc.sync.dma_start(
                        out=qt_h, in_=qt.ap()[s, :, h * HALF:(h + 1) * HALF]
                    )
                    sb_qt.append(qt_h)
                    if h == 0:
                        sb_kt = io.tile([D, n_c, CH], BF16, tag="kt")
                        nc.sync.dma_start(
                            out=sb_kt,
                            in_=kt.ap()[s, :, 0:n_c * CH].rearrange(
                                "d (c p) -> d c p", p=CH
                            ),
                        )
                if s == slot_order[-1]:
                    ps_ot = ps_sc_pool.tile([V, Q], F32, tag="sc", name="ps_ot_last")
                else:
                    ps_ot = ps_ot_pool.tile([V, Q], F32, tag="ot")
                acc = accp.tile([CH, Q], BF16, tag="acc")
                # Software-pipelined: chunk j's AV matmuls are emitted one
                # iteration late so PE never stalls on the current exp.
                exp_tiles = {}
                for j in range(n_c + 1):
                    tail = (s == slot_order[-1])
                    if j < n_c:
                        ktj = sb_kt[:, j, :]
                        ps_scores = ps_sc_pool.tile([CH, Q], F32, tag="sc")
                        for h in range(2):
                            nc.tensor.matmul(
                                ps_scores[:, h * HALF:(h + 1) * HALF],
                                lhsT=ktj, rhs=sb_qt[h],
                                start=True, stop=True,
                            )
                        sb_exp = expp.tile([CH, Q], BF16, tag="exp")
                        nc.scalar.activation(
                            sb_exp, ps_scores,
                            func=mybir.ActivationFunctionType.Exp,
                            bias=bias_t[:, s, j:j + 1],
                            scale=SCALE,
                        )
                        exp_tiles[j] = sb_exp
                    if j >= 1:
                        jj = j - 1
                        e = exp_tiles.pop(jj)
                        vj = sb_v[:, jj, :]
                        for h in range(2):
                            hs = slice(h * HALF, (h + 1) * HALF)
                            eh = e[h] if isinstance(e, tuple) else e[:, hs]
                            nc.tensor.matmul(
                                ps_ot[:, hs], lhsT=vj, rhs=eh,
                                start=(jj == 0), stop=(jj == n_c - 1),
                            )
                        if isinstance(e, tuple):
                            for h in range(2):
                                hs = slice(h * HALF, (h + 1) * HALF)
                                nc.vector.tensor_add(acc[:, hs], acc[:, hs], e[h])
                        elif jj == 0:
                            nc.vector.tensor_copy(acc, e)
                        else:
                            nc.vector.tensor_add(acc, acc, e)
                nc.gpsimd.dma_start(out=am.ap()[s], in_=acc)
                for h in range(2):
                    hs = slice(h * HALF, (h + 1) * HALF)
                    sb_ot = outp.tile([V, HALF], BF16, tag=f"ot{h}")
                    if h == 1 and s == slot_order[-1]:
                        nc.scalar.copy(sb_ot, ps_ot[:, hs])
                    else:
                        nc.vector.tensor_copy(sb_ot, ps_ot[:, hs])
                    nc.sync.dma_start(out=ot.ap()[s, :, hs], in_=sb_ot)
    nc.compile()
    return nc


_MODULE_CACHE = {}


def _get_module(n_chunks):
    key = tuple(n_chunks)
    if key not in _MODULE_CACHE:
        _MODULE_CACHE[key] = _build(key)
    return _MODULE_CACHE[key]


def _plan(L):
    """Assign batches to (slot, core) sorted by valid_len; per-slot chunk count."""
    order = np.argsort(L, kind="stable")
    grid = order.reshape(S, N_CORES)       # grid[s, c] = batch index
    n_chunks = tuple(
        max(1, int(math.ceil(int(L[grid[s, -1]]) / CH))) for s in range(S)
    )
    return grid, n_chunks


def _prepare_inputs(q, k, v, L, grid):
    kidx = np.arange(K).reshape(NCH, CH).T      # [CH, NCH] k index per (p, chunk)
    in_maps = []
    for c in range(N_CORES):
        bs = grid[:, c]
        qt_c = np.ascontiguousarray(q[bs].transpose(0, 2, 1)).astype(ml_dtypes.bfloat16)
        kt_c = np.ascontiguousarray(k[bs].transpose(0, 2, 1)).astype(ml_dtypes.bfloat16)
        v_c = np.ascontiguousarray(v[bs]).astype(ml_dtypes.bfloat16)
        mb_c = np.empty((S, CH, NCH + 1), np.float32)
        mb_c[:, :, NCH] = 1.0
        for s in range(S):
            mb_c[s, :, :NCH] = np.where(kidx < int(L[grid[s, c]]), 0.0, NEG_BIAS)
        in_maps.append({"qt": qt_c, "kt": kt_c, "v": v_c, "mbias": mb_c})
    return in_maps


def _postprocess(results, grid):
    out = np.empty((B, Q, V), np.float32)
    for c in range(N_CORES):
        otc = results[c]["ot"]
        sums = results[c]["acc"].astype(np.float32).sum(axis=1)  # [S, Q]
        for s in range(S):
            b = grid[s, c]
            out[b] = otc[s].T / sums[s][:, None]
    return out


def kernel(**inputs):
    q = np.ascontiguousarray(np.asarray(inputs["queries"], dtype=np.float32))
    k = np.ascontiguousarray(np.asarray(inputs["keys"], dtype=np.float32))
    v = np.ascontiguousarray(np.asarray(inputs["values"], dtype=np.float32))
    L = np.clip(np.asarray(inputs["valid_lens"]).astype(np.int64).reshape(-1), 1, K)
    grid, n_chunks = _plan(L)
    nc = _get_module(n_chunks)
    in_maps = _prepare_inputs(q, k, v, L, grid)
    res = run_bass_kernel_spmd(nc, in_maps, core_ids=list(range(N_CORES)))
    return _postprocess(res.results, grid)
